# revision 23
# baseline (speedup 1.0000x reference)
"""Multi-head causal attention on 8 TRN2 NeuronCores.

Sharding: core c -> (b = c // 4, hg = c % 4). Data parallel over the batch
dim (B=2), tensor parallel over heads (16 heads -> 4 groups of 4). Each core
computes q/k/v projections for its 4 heads on its batch row, full causal
attention for those heads, and a partial output projection through its
256-row slice of Wp. The host sums the 4 head-group partials per batch
(the tensor-parallel reduce) and adds the output bias.

The kernel is a fully cascaded stage pipeline over 512-row stages t4=0..3
(all matmuls bf16 with fp32 PSUM accumulation):
  load(t4)   x rows -> SBUF, cast bf16             (one stage ahead)
  build(t4)  TensorE transposes -> xT chunk; qT/kT chunk projections;
             v rows with a per-head ones column
  attn(qc)   scoresT = k q^T in [keys, q] tiles (two key blocks share one
             2-bank PSUM tile and one ScalarE exp; scale 1/8 folded into the
             exp, no max subtraction -- scores are O(3)); causal mask via a
             0/1 triangular-mask multiply on DVE; PV matmul with [v | 1]
             stationary yields y^T plus the softmax denominator row;
             normalize via reciprocal_approx_fast + gpsimd partition
             broadcast. The scores stream runs ~4 key blocks ahead of the
             PV stream so the PE never waits on exp.
  out(qc)    out = y @ Wp_s via yT-stationary matmuls, DMA partials out,
             interleaved into the next stage's attention stream.
"""

import numpy as np

import concourse.bass as bass
import concourse.mybir as mybir
import concourse.tile as tile
from concourse import bacc
from concourse.bass_utils import run_bass_kernel_spmd
from concourse.masks import make_identity

F32 = mybir.dt.float32
BF16 = mybir.dt.bfloat16

B, T, C, H = 2, 2048, 1024, 16
NCORES = 8
HG = 4            # head groups (tensor-parallel degree)
NH = H // HG      # heads per core = 4
HD = C // H       # head dim = 64
HS = NH * HD      # head-slice width per core = 256
SCALE = 1.0 / float(np.sqrt(HD))

TB = T // 128     # 16 row blocks
CCH = C // 128    # 8 contraction chunks
QC = T // 512     # 4 q chunks of 512


def _body(tc):
    nc = tc.nc
    x = nc.dram_tensor("x", [T, C], F32, kind="ExternalInput").ap()
    wq = nc.dram_tensor("wq", [C, HS], F32, kind="ExternalInput").ap()
    wk = nc.dram_tensor("wk", [C, HS], F32, kind="ExternalInput").ap()
    wv = nc.dram_tensor("wv", [C, HS], F32, kind="ExternalInput").ap()
    wp = nc.dram_tensor("wp", [HS, C], F32, kind="ExternalInput").ap()
    bq = nc.dram_tensor("bq", [HS], F32, kind="ExternalInput").ap()
    bk = nc.dram_tensor("bk", [HS], F32, kind="ExternalInput").ap()
    bv = nc.dram_tensor("bv", [HS], F32, kind="ExternalInput").ap()
    bp = nc.dram_tensor("bp", [C], F32, kind="ExternalInput").ap()
    out = nc.dram_tensor("out", [T, C], F32, kind="ExternalOutput").ap()

    with (
        tc.tile_pool(name="const", bufs=1) as const,
        tc.tile_pool(name="persist", bufs=1) as persist,
        tc.tile_pool(name="xinp", bufs=5) as xinp,
        tc.tile_pool(name="xbfp", bufs=8) as xbfp,
        tc.tile_pool(name="stage", bufs=2) as stage,
        tc.tile_pool(name="work", bufs=3) as work,
        tc.tile_pool(name="expp", bufs=4) as expp,
        tc.tile_pool(name="mmps", bufs=2, space="PSUM") as mmps,
        tc.tile_pool(name="sps2", bufs=2, space="PSUM") as spsp,
        tc.tile_pool(name="yps", bufs=2, space="PSUM") as ypsp,
    ):
        # HAM warmup first: the clock gate watches MAC activity, so these
        # must be full-K matmuls, and their memset must be the first gpsimd op
        warm_in = const.tile([128, 512], BF16, tag="warm_in")
        nc.gpsimd.memset(warm_in[:], 0.0)
        for r in range(12):
            wps = mmps.tile([128, 512], F32, tag="mm512", name=f"warm{r}")
            nc.tensor.matmul(wps[:], warm_in[:, :128], warm_in[:], start=True, stop=True)

        ident = const.tile([128, 128], BF16, tag="ident")
        make_identity(nc, ident[:])
        ones1 = const.tile([1, 128], BF16, tag="ones1")
        nc.gpsimd.memset(ones1[:], 1.0)
        # 0/1 lower-triangular mask (keep (i, j) iff j >= i) for the
        # diagonal 128-col strips, applied post-exp as a DVE multiply
        trimask = const.tile([128, 128], BF16, tag="trimask")
        nc.gpsimd.memset(trimask[:], 1.0)
        nc.gpsimd.affine_select(
            out=trimask[:], in_=trimask[:],
            compare_op=mybir.AluOpType.is_ge,
            fill=0.0, base=0, pattern=[[1, 128]], channel_multiplier=-1,
        )

        # ---- weights + biases (DMA on the gpsimd queue so the x-tile DMAs
        # on the sync queue are not delayed) ----------------------------
        wq_b = persist.tile([128, CCH, HS], BF16, tag="wq_b")
        wk_b = persist.tile([128, CCH, HS], BF16, tag="wk_b")
        wv_b = persist.tile([128, CCH, HS], BF16, tag="wv_b")
        wp_b = persist.tile([128, HS // 128, C], BF16, tag="wp_b")
        for dst, src in ((wq_b, wq), (wk_b, wk), (wv_b, wv)):
            wf = stage.tile([128, CCH, HS], F32, tag="wstage")
            nc.gpsimd.dma_start(wf[:], src.rearrange("(o p) n -> p o n", p=128))
            nc.vector.tensor_copy(dst[:], wf[:])
        wpf = stage.tile([128, HS // 128, C], F32, tag="wstage")
        nc.gpsimd.dma_start(wpf[:], wp.rearrange("(o p) n -> p o n", p=128))
        nc.vector.tensor_copy(wp_b[:], wpf[:])

        bq_sb = const.tile([128, 2], F32, tag="bq_sb")
        nc.gpsimd.dma_start(bq_sb[:], bq.rearrange("(o p) -> p o", p=128))
        bk_sb = const.tile([128, 2], F32, tag="bk_sb")
        nc.gpsimd.dma_start(bk_sb[:], bk.rearrange("(o p) -> p o", p=128))

        # bv broadcast across partitions via ones outer product
        bv_row = const.tile([1, HS], F32, tag="bv_row")
        nc.gpsimd.dma_start(bv_row[:], bv.rearrange("(o n) -> o n", o=1))
        bv_rowb = const.tile([1, HS], BF16, tag="bv_rowb")
        nc.vector.tensor_copy(bv_rowb[:], bv_row[:])
        bv_bc = persist.tile([128, HS], F32, tag="bv_bc")
        ps = mmps.tile([128, 512], F32, tag="mm512")
        nc.tensor.matmul(ps[:, :HS], ones1[:], bv_rowb[:], start=True, stop=True)
        nc.vector.tensor_copy(bv_bc[:], ps[:, :HS])

        # ---- persistent stage outputs ---------------------------------
        xT = [persist.tile([128, CCH, 512], BF16, tag=f"xT{t4}", name=f"xT{t4}")
              for t4 in range(QC)]
        qTc = [[persist.tile([128, 512], BF16, tag=f"qTc{p}_{t}", name=f"qTc{p}_{t}")
                for t in range(QC)] for p in range(2)]
        kTc = [[persist.tile([128, 512], BF16, tag=f"kTc{p}_{t}", name=f"kTc{p}_{t}")
                for t in range(QC)] for p in range(2)]
        v_sb = [persist.tile([128, 4, NH * 65], BF16, tag=f"v_sb{i}",
                             name=f"v_sb{i}") for i in range(4)]
        for i in range(4):
            nc.gpsimd.memset(
                v_sb[i][:].rearrange("p k (h e) -> p k h e", e=65)[:, :, :, 64:65], 1.0
            )
        yT = [persist.tile([128, 512], BF16, tag=f"yT{q}", name=f"yT{q}")
              for q in range(QC * 2)]  # index 2*qc + pair

        xbfs = {}

        def s0_load(t4):
            # x rows in + bf16 cast; issued one stage ahead of use
            for tb in range(4 * t4, 4 * t4 + 4):
                xin = xinp.tile([128, C], F32, tag="xin", name=f"xin{tb}")
                nc.sync.dma_start(xin[:], x[tb * 128 : (tb + 1) * 128, :])
                xbf = xbfp.tile([128, C], BF16, tag="xbf", name=f"xbf{tb}")
                if tb % 2 == 0:
                    nc.scalar.copy(xbf[:], xin[:])
                else:
                    nc.vector.tensor_copy(xbf[:], xin[:])
                xbfs[tb] = xbf

        def s0_transpose(t4):
            for tb in range(4 * t4, 4 * t4 + 4):
                xbf = xbfs.pop(tb)
                trem = tb % 4
                for cc in range(CCH):
                    tps = mmps.tile([128, 512], BF16, tag="mm512")
                    nc.tensor.transpose(
                        tps[:, :128], xbf[:, cc * 128 : (cc + 1) * 128], ident[:]
                    )
                    dst = xT[t4][:, cc, trem * 128 : (trem + 1) * 128]
                    if cc % 2 == 0:
                        nc.scalar.copy(dst, tps[:, :128])
                    else:
                        nc.vector.tensor_copy(dst, tps[:, :128])

        def qk_chunk(t4):
            for pair in range(2):
                for which, w_b, b_sb in (("q", wq_b, bq_sb), ("k", wk_b, bk_sb)):
                    ps = mmps.tile([128, 512], F32, tag="mm512",
                                   name=f"{which}ps{pair}_{t4}")
                    for cc in range(CCH):
                        nc.tensor.matmul(
                            ps[:],
                            w_b[:, cc, pair * 128 : (pair + 1) * 128],
                            xT[t4][:, cc, :],
                            start=(cc == 0),
                            stop=(cc == CCH - 1),
                        )
                    if which == "q":
                        nc.scalar.activation(
                            qTc[pair][t4][:], ps[:],
                            mybir.ActivationFunctionType.Identity,
                            bias=b_sb[:, pair : pair + 1], scale=1.0,
                        )
                    else:
                        nc.vector.tensor_scalar_add(
                            kTc[pair][t4][:], ps[:], b_sb[:, pair : pair + 1]
                        )

        def v_group(g):
            for tb in range(4 * g, 4 * g + 4):
                ps = mmps.tile([128, 512], F32, tag="mm512", name=f"vps{tb}")
                for cc in range(CCH):
                    nc.tensor.matmul(
                        ps[:, :HS],
                        xT[tb // 4][:, cc, (tb % 4) * 128 : (tb % 4 + 1) * 128],
                        wv_b[:, cc, :],
                        start=(cc == 0),
                        stop=(cc == CCH - 1),
                    )
                vdst = v_sb[tb // 4][:, tb % 4, :].rearrange(
                    "p (h e) -> p h e", e=65)[:, :, 0:64]
                nc.vector.tensor_tensor(vdst, ps[:, :HS], bv_bc[:], mybir.AluOpType.add)

        # ---- attention + output, software pipelined -------------------
        units = []  # (h, qc, kb, is_last)
        for qc in range(QC):
            for h in range(NH):
                nkb = 4 * qc + 4
                for kb in range(nkb):
                    units.append((h, qc, kb, kb == nkb - 1))
        esbs = {}
        yps_tiles = {}

        def emit_scores_pair(i):
            # scores + exp for units i and i+1 (same h/qc, kb even/odd pair)
            h, qc, kb0, _ = units[i]
            pair, off = h // 2, 64 * (h % 2)
            d0 = max(0, 128 * (kb0 - 4 * qc))
            d1 = max(0, 128 * (kb0 + 1 - 4 * qc))
            sps = spsp.tile([128, 2, 512], F32, tag="sps2", name=f"sps{i}")
            esb = expp.tile([128, 2, 512], BF16, tag="esb", name=f"esb{i}")
            for j, d in ((0, d0), (1, d1)):
                kb = kb0 + j
                nc.tensor.matmul(
                    sps[:, j, d:512],
                    kTc[pair][kb // 4][off : off + 64,
                                       (kb % 4) * 128 : (kb % 4 + 1) * 128],
                    qTc[pair][qc][off : off + 64, d:512],
                    start=True, stop=True,
                )
            # one exp covers both halves when the pair is uniform; diagonal
            # pairs split in two so no unwritten PSUM is read
            flat_s = sps[:].rearrange("p a b -> p (a b)")
            flat_e = esb[:].rearrange("p a b -> p (a b)")
            if d0 == d1:
                nc.scalar.activation(
                    flat_e[:, d0:1024], flat_s[:, d0:1024],
                    mybir.ActivationFunctionType.Exp, scale=SCALE,
                )
            else:
                nc.scalar.activation(
                    flat_e[:, d0:512], flat_s[:, d0:512],
                    mybir.ActivationFunctionType.Exp, scale=SCALE,
                )
                nc.scalar.activation(
                    flat_e[:, 512 + d1 : 1024], flat_s[:, 512 + d1 : 1024],
                    mybir.ActivationFunctionType.Exp, scale=SCALE,
                )
            for j, d in ((0, d0), (1, d1)):
                if units[i + j][2] >= 4 * qc:
                    # zero the upper triangle of the diagonal 128-col strip
                    nc.vector.tensor_tensor(
                        esb[:, j, d : d + 128], esb[:, j, d : d + 128],
                        trimask[:], mybir.AluOpType.mult,
                    )
            esbs[i] = esb
            esbs[i + 1] = esb

        def emit_pv(i):
            h, qc, kb, is_last = units[i]
            pair, off = h // 2, 64 * (h % 2)
            d = max(0, 128 * (kb - 4 * qc))
            if kb == 0:
                yps_tiles[(h, qc)] = ypsp.tile(
                    [65, 512], F32, tag="yps", name=f"yps{h}_{qc}"
                )
            yps = yps_tiles[(h, qc)]
            nc.tensor.matmul(
                yps[:, d:512],
                v_sb[kb // 4][:, kb % 4, 65 * h : 65 * h + 65],
                esbs.pop(i)[:, kb % 2, d:512],
                start=(kb == 0),
                stop=is_last,
            )
            if not is_last:
                return
            # normalize: row 64 of yps is the softmax denominator
            den = work.tile([1, 512], F32, tag="den")
            nc.vector.tensor_copy(den[:], yps[64:65, :])
            rec = work.tile([1, 512], F32, tag="rec")
            nc.vector.reciprocal_approx_fast(rec[:], den[:])
            rbc = work.tile([64, 512], F32, tag="rbc")
            nc.gpsimd.partition_broadcast(rbc[:], rec[:])
            nc.vector.tensor_tensor(
                yT[2 * qc + pair][off : off + 64, :],
                yps[0:64, :], rbc[:], mybir.AluOpType.mult,
            )

        def emit_s4(qc):
            for qb in range(4 * qc, 4 * qc + 4):
                osb = work.tile([128, C], F32, tag="osb", name=f"osb{qb}")
                for cc2 in range(2):
                    ps = mmps.tile([128, 512], F32, tag="mm512", name=f"ops{qb}_{cc2}")
                    for ych in range(HS // 128):
                        nc.tensor.matmul(
                            ps[:],
                            yT[2 * qc + ych][:, (qb % 4) * 128 : (qb % 4 + 1) * 128],
                            wp_b[:, ych, cc2 * 512 : (cc2 + 1) * 512],
                            start=(ych == 0),
                            stop=(ych == HS // 128 - 1),
                        )
                    dst = osb[:, cc2 * 512 : (cc2 + 1) * 512]
                    if cc2 == 0:
                        nc.scalar.copy(dst, ps[:])
                    else:
                        nc.vector.tensor_copy(dst, ps[:])
                nc.sync.dma_start(out[qb * 128 : (qb + 1) * 128, :], osb[:])

        LOOKAHEAD = 4
        scores_done = 0
        loaded = 0
        built = 0
        pending_s4 = []

        def ensure_stage(t4):
            nonlocal loaded, built
            while loaded <= min(t4 + 1, QC - 1):
                s0_load(loaded)
                loaded += 1
            while built <= t4:
                s0_transpose(built)
                qk_chunk(built)
                v_group(built)
                built += 1

        def advance_scores(target):
            nonlocal scores_done
            while scores_done < min(target, len(units)):
                ensure_stage(units[scores_done][1])
                emit_scores_pair(scores_done)
                scores_done += 2

        for i in range(len(units)):
            advance_scores(i + 1 + LOOKAHEAD)
            emit_pv(i)
            if pending_s4:
                emit_s4(pending_s4.pop())
            h, qc, kb, is_last = units[i]
            if is_last and h == NH - 1:
                if i == len(units) - 1:
                    emit_s4(qc)
                else:
                    pending_s4.append(qc)


_NC = None


def _build():
    global _NC
    if _NC is None:
        nc = bacc.Bacc("TRN2", target_bir_lowering=False)
        with tile.TileContext(nc) as tc:
            _body(tc)
        nc.compile()
        _NC = nc
    return _NC


def _shard_inputs(x, Wq, bq, Wk, bk, Wv, bv, Wp, bp):
    f = lambda a: np.ascontiguousarray(np.asarray(a, dtype=np.float32))
    zc = np.zeros(C, np.float32)
    in_maps = []
    for c in range(NCORES):
        b, hg = divmod(c, HG)
        cols = slice(hg * HS, (hg + 1) * HS)
        in_maps.append({
            "x": f(x[b]),
            "wq": f(Wq[:, cols]), "wk": f(Wk[:, cols]), "wv": f(Wv[:, cols]),
            "wp": f(Wp[cols, :]),
            "bq": f(bq[cols]), "bk": f(bk[cols]), "bv": f(bv[cols]),
            # bp is applied host-side during the unshard reduce
            "bp": zc,
        })
    return in_maps


def run_sharded(inputs, **run_kwargs):
    """Compile (cached), run on cores 0-7, gather. Returns (out, results)."""
    nc = _build()
    in_maps = _shard_inputs(**inputs)
    res = run_bass_kernel_spmd(nc, in_maps, core_ids=list(range(NCORES)), **run_kwargs)
    out = np.zeros((B, T, C), np.float32)
    for c in range(NCORES):
        b = c // HG
        out[b] += res.results[c]["out"]
    out += np.asarray(inputs["bp"], dtype=np.float32)
    return out, res


def kernel(x, Wq, bq, Wk, bk, Wv, bv, Wp, bp):
    out, _ = run_sharded(dict(
        x=x, Wq=Wq, bq=bq, Wk=Wk, bk=bk, Wv=Wv, bv=bv, Wp=Wp, bp=bp,
    ))
    return out


# revision 24
# speedup vs baseline: 1.1853x; 1.1853x over previous
"""Multi-head causal attention on 8 TRN2 NeuronCores.

Sharding: core c -> (b = c // 4, hg = c % 4). Data parallel over the batch
dim (B=2), tensor parallel over heads (16 heads -> 4 groups of 4). Each core
computes q/k/v projections for its 4 heads on its batch row, full causal
attention for those heads, and a partial output projection through its
256-row slice of Wp. The host sums the 4 head-group partials per batch
(the tensor-parallel reduce) and adds the output bias.

The kernel is a fully cascaded stage pipeline over 512-row stages t4=0..3
(all matmuls bf16 with fp32 PSUM accumulation):
  load(t4)   x rows -> SBUF, cast bf16             (one stage ahead)
  build(t4)  TensorE transposes -> xT chunk; qT/kT chunk projections;
             v rows with a per-head ones column
  attn(qc)   scoresT = k q^T in [keys, q] tiles (two key blocks share one
             2-bank PSUM tile and one ScalarE exp; scale 1/8 folded into the
             exp, no max subtraction -- scores are O(3)); causal mask via a
             0/1 triangular-mask multiply on DVE; PV matmul with [v | 1]
             stationary yields y^T plus the softmax denominator row;
             normalize via reciprocal_approx_fast + gpsimd partition
             broadcast. The scores stream runs ~4 key blocks ahead of the
             PV stream so the PE never waits on exp.
  out(qc)    out = y @ Wp_s via yT-stationary matmuls, DMA partials out,
             interleaved into the next stage's attention stream.
"""

import numpy as np

import concourse.bass as bass
import concourse.mybir as mybir
import concourse.tile as tile
from concourse import bacc
from concourse.bass_utils import run_bass_kernel_spmd
from concourse.masks import make_identity

F32 = mybir.dt.float32
BF16 = mybir.dt.bfloat16

B, T, C, H = 2, 2048, 1024, 16
NCORES = 8
HG = 4            # head groups (tensor-parallel degree)
NH = H // HG      # heads per core = 4
HD = C // H       # head dim = 64
HS = NH * HD      # head-slice width per core = 256
SCALE = 1.0 / float(np.sqrt(HD))

TB = T // 128     # 16 row blocks
CCH = C // 128    # 8 contraction chunks
QC = T // 512     # 4 q chunks of 512


def _body(tc):
    nc = tc.nc
    x = nc.dram_tensor("x", [T, C], F32, kind="ExternalInput").ap()
    wq = nc.dram_tensor("wq", [C, HS], F32, kind="ExternalInput").ap()
    wk = nc.dram_tensor("wk", [C, HS], F32, kind="ExternalInput").ap()
    wv = nc.dram_tensor("wv", [C, HS], F32, kind="ExternalInput").ap()
    wp = nc.dram_tensor("wp", [HS, C], F32, kind="ExternalInput").ap()
    bq = nc.dram_tensor("bq", [HS], F32, kind="ExternalInput").ap()
    bk = nc.dram_tensor("bk", [HS], F32, kind="ExternalInput").ap()
    bv = nc.dram_tensor("bv", [HS], F32, kind="ExternalInput").ap()
    bp = nc.dram_tensor("bp", [C], F32, kind="ExternalInput").ap()
    out = nc.dram_tensor("out", [T, C], F32, kind="ExternalOutput").ap()

    with (
        tc.tile_pool(name="const", bufs=1) as const,
        tc.tile_pool(name="persist", bufs=1) as persist,
        tc.tile_pool(name="xinp", bufs=5) as xinp,
        tc.tile_pool(name="xbfp", bufs=8) as xbfp,
        tc.tile_pool(name="stage", bufs=2) as stage,
        tc.tile_pool(name="work", bufs=3) as work,
        tc.tile_pool(name="expp", bufs=4) as expp,
        tc.tile_pool(name="mmps", bufs=2, space="PSUM") as mmps,
        tc.tile_pool(name="sps2", bufs=2, space="PSUM") as spsp,
        tc.tile_pool(name="yps", bufs=2, space="PSUM") as ypsp,
    ):
        ident = const.tile([128, 128], BF16, tag="ident")
        make_identity(nc, ident[:])
        ones1 = const.tile([1, 128], BF16, tag="ones1")
        nc.gpsimd.memset(ones1[:], 1.0)
        # 0/1 lower-triangular mask (keep (i, j) iff j >= i) for the
        # diagonal 128-col strips, applied post-exp as a DVE multiply
        trimask = const.tile([128, 128], BF16, tag="trimask")
        nc.gpsimd.memset(trimask[:], 1.0)
        nc.gpsimd.affine_select(
            out=trimask[:], in_=trimask[:],
            compare_op=mybir.AluOpType.is_ge,
            fill=0.0, base=0, pattern=[[1, 128]], channel_multiplier=-1,
        )

        # ---- S0: x load + transpose + cast ----------------------------
        xT = [persist.tile([128, CCH, 512], BF16, tag=f"xT{t4}", name=f"xT{t4}")
              for t4 in range(QC)]
        for tb in range(TB):
            xin = xinp.tile([128, C], F32, tag="xin", name=f"xin{tb}")
            nc.sync.dma_start(xin[:], x[tb * 128 : (tb + 1) * 128, :])
            xbf = xbfp.tile([128, C], BF16, tag="xbf", name=f"xbf{tb}")
            if tb % 2 == 0:
                nc.scalar.copy(xbf[:], xin[:])
            else:
                nc.vector.tensor_copy(xbf[:], xin[:])
            t4, trem = divmod(tb, 4)
            for cc in range(CCH):
                tps = mmps.tile([128, 512], BF16, tag="mm512")
                nc.tensor.transpose(
                    tps[:, :128], xbf[:, cc * 128 : (cc + 1) * 128], ident[:]
                )
                dst = xT[t4][:, cc, trem * 128 : (trem + 1) * 128]
                if cc % 2 == 0:
                    nc.scalar.copy(dst, tps[:, :128])
                else:
                    nc.vector.tensor_copy(dst, tps[:, :128])

        # ---- S1: weights + biases -------------------------------------
        wq_b = persist.tile([128, CCH, HS], BF16, tag="wq_b")
        wk_b = persist.tile([128, CCH, HS], BF16, tag="wk_b")
        wv_b = persist.tile([128, CCH, HS], BF16, tag="wv_b")
        wp_b = persist.tile([128, HS // 128, C], BF16, tag="wp_b")
        for dst, src in ((wq_b, wq), (wk_b, wk), (wv_b, wv)):
            wf = stage.tile([128, CCH, HS], F32, tag="wstage")
            nc.sync.dma_start(wf[:], src.rearrange("(o p) n -> p o n", p=128))
            nc.vector.tensor_copy(dst[:], wf[:])
        wpf = stage.tile([128, HS // 128, C], F32, tag="wstage")
        nc.sync.dma_start(wpf[:], wp.rearrange("(o p) n -> p o n", p=128))
        nc.vector.tensor_copy(wp_b[:], wpf[:])

        bq_sb = const.tile([128, 2], F32, tag="bq_sb")
        nc.sync.dma_start(bq_sb[:], bq.rearrange("(o p) -> p o", p=128))
        bk_sb = const.tile([128, 2], F32, tag="bk_sb")
        nc.sync.dma_start(bk_sb[:], bk.rearrange("(o p) -> p o", p=128))

        # bv broadcast across partitions via ones outer product
        bv_row = const.tile([1, HS], F32, tag="bv_row")
        nc.sync.dma_start(bv_row[:], bv.rearrange("(o n) -> o n", o=1))
        bv_rowb = const.tile([1, HS], BF16, tag="bv_rowb")
        nc.vector.tensor_copy(bv_rowb[:], bv_row[:])
        bv_bc = persist.tile([128, HS], F32, tag="bv_bc")
        ps = mmps.tile([128, 512], F32, tag="mm512")
        nc.tensor.matmul(ps[:, :HS], ones1[:], bv_rowb[:], start=True, stop=True)
        nc.vector.tensor_copy(bv_bc[:], ps[:, :HS])

        # ---- S2: q/k projections (pair-major so attention starts early) ---
        qT = [persist.tile([128, T], BF16, tag=f"qT{p}", name=f"qT{p}") for p in range(2)]
        kT = [persist.tile([128, T], BF16, tag=f"kT{p}", name=f"kT{p}") for p in range(2)]
        for pair in range(2):
            for dstT, w_b, b_sb, eng in (
                (qT, wq_b, bq_sb, "scalar"),
                (kT, wk_b, bk_sb, "vector"),
            ):
                for t4 in range(QC):
                    ps = mmps.tile([128, 512], F32, tag="mm512")
                    for cc in range(CCH):
                        nc.tensor.matmul(
                            ps[:],
                            w_b[:, cc, pair * 128 : (pair + 1) * 128],
                            xT[t4][:, cc, :],
                            start=(cc == 0),
                            stop=(cc == CCH - 1),
                        )
                    dst = dstT[pair][:, t4 * 512 : (t4 + 1) * 512]
                    if eng == "scalar":
                        nc.scalar.activation(
                            dst, ps[:], mybir.ActivationFunctionType.Identity,
                            bias=b_sb[:, pair : pair + 1], scale=1.0,
                        )
                    else:
                        nc.vector.tensor_scalar_add(dst, ps[:], b_sb[:, pair : pair + 1])

        # v in natural layout [T, 4 heads x (64 + ones col)]; 4 tiles of 4 row
        # blocks each, emitted just-in-time inside the attention stream
        v_sb = [persist.tile([128, 4, NH * 65], BF16, tag=f"v_sb{i}",
                             name=f"v_sb{i}") for i in range(4)]
        for i in range(4):
            nc.gpsimd.memset(
                v_sb[i][:].rearrange("p k (h e) -> p k h e", e=65)[:, :, :, 64:65], 1.0
            )
        yT = [persist.tile([128, 512], BF16, tag=f"yT{q}", name=f"yT{q}")
              for q in range(QC * 2)]  # index 2*qc + pair

        def v_group(g):
            for tb in range(4 * g, 4 * g + 4):
                ps = mmps.tile([128, 512], F32, tag="mm512", name=f"vps{tb}")
                for cc in range(CCH):
                    nc.tensor.matmul(
                        ps[:, :HS],
                        xT[tb // 4][:, cc, (tb % 4) * 128 : (tb % 4 + 1) * 128],
                        wv_b[:, cc, :],
                        start=(cc == 0),
                        stop=(cc == CCH - 1),
                    )
                vdst = v_sb[tb // 4][:, tb % 4, :].rearrange(
                    "p (h e) -> p h e", e=65)[:, :, 0:64]
                nc.vector.tensor_tensor(vdst, ps[:, :HS], bv_bc[:], mybir.AluOpType.add)

        # ---- attention + output, software pipelined -------------------
        units = []  # (h, qc, kb, is_last)
        for qc in range(QC):
            for h in range(NH):
                nkb = 4 * qc + 4
                for kb in range(nkb):
                    units.append((h, qc, kb, kb == nkb - 1))
        esbs = {}
        yps_tiles = {}

        def emit_scores_pair(i):
            # scores + exp for units i and i+1 (same h/qc, kb even/odd pair)
            h, qc, kb0, _ = units[i]
            pair, off = h // 2, 64 * (h % 2)
            d0 = max(0, 128 * (kb0 - 4 * qc))
            d1 = max(0, 128 * (kb0 + 1 - 4 * qc))
            sps = spsp.tile([128, 2, 512], F32, tag="sps2", name=f"sps{i}")
            esb = expp.tile([128, 2, 512], BF16, tag="esb", name=f"esb{i}")
            for j, d in ((0, d0), (1, d1)):
                kb = kb0 + j
                nc.tensor.matmul(
                    sps[:, j, d:512],
                    kT[pair][off : off + 64, kb * 128 : (kb + 1) * 128],
                    qT[pair][off : off + 64, qc * 512 + d : (qc + 1) * 512],
                    start=True, stop=True,
                )
            # one exp covers both halves when the pair is uniform; diagonal
            # pairs split in two so no unwritten PSUM is read
            flat_s = sps[:].rearrange("p a b -> p (a b)")
            flat_e = esb[:].rearrange("p a b -> p (a b)")
            if d0 == d1:
                nc.scalar.activation(
                    flat_e[:, d0:1024], flat_s[:, d0:1024],
                    mybir.ActivationFunctionType.Exp, scale=SCALE,
                )
            else:
                nc.scalar.activation(
                    flat_e[:, d0:512], flat_s[:, d0:512],
                    mybir.ActivationFunctionType.Exp, scale=SCALE,
                )
                nc.scalar.activation(
                    flat_e[:, 512 + d1 : 1024], flat_s[:, 512 + d1 : 1024],
                    mybir.ActivationFunctionType.Exp, scale=SCALE,
                )
            for j, d in ((0, d0), (1, d1)):
                if units[i + j][2] >= 4 * qc:
                    # zero the upper triangle of the diagonal 128-col strip
                    nc.vector.tensor_tensor(
                        esb[:, j, d : d + 128], esb[:, j, d : d + 128],
                        trimask[:], mybir.AluOpType.mult,
                    )
            esbs[i] = esb
            esbs[i + 1] = esb

        def emit_pv(i):
            h, qc, kb, is_last = units[i]
            pair, off = h // 2, 64 * (h % 2)
            d = max(0, 128 * (kb - 4 * qc))
            if kb == 0:
                yps_tiles[(h, qc)] = ypsp.tile(
                    [65, 512], F32, tag="yps", name=f"yps{h}_{qc}"
                )
            yps = yps_tiles[(h, qc)]
            nc.tensor.matmul(
                yps[:, d:512],
                v_sb[kb // 4][:, kb % 4, 65 * h : 65 * h + 65],
                esbs.pop(i)[:, kb % 2, d:512],
                start=(kb == 0),
                stop=is_last,
            )
            if not is_last:
                return
            # normalize: row 64 of yps is the softmax denominator
            den = work.tile([1, 512], F32, tag="den")
            nc.vector.tensor_copy(den[:], yps[64:65, :])
            rec = work.tile([1, 512], F32, tag="rec")
            nc.vector.reciprocal_approx_fast(rec[:], den[:])
            rbc = work.tile([64, 512], F32, tag="rbc")
            nc.gpsimd.partition_broadcast(rbc[:], rec[:])
            nc.vector.tensor_tensor(
                yT[2 * qc + pair][off : off + 64, :],
                yps[0:64, :], rbc[:], mybir.AluOpType.mult,
            )

        def emit_s4(qc):
            for qb in range(4 * qc, 4 * qc + 4):
                osb = work.tile([128, C], F32, tag="osb", name=f"osb{qb}")
                for cc2 in range(2):
                    ps = mmps.tile([128, 512], F32, tag="mm512", name=f"ops{qb}_{cc2}")
                    for ych in range(HS // 128):
                        nc.tensor.matmul(
                            ps[:],
                            yT[2 * qc + ych][:, (qb % 4) * 128 : (qb % 4 + 1) * 128],
                            wp_b[:, ych, cc2 * 512 : (cc2 + 1) * 512],
                            start=(ych == 0),
                            stop=(ych == HS // 128 - 1),
                        )
                    dst = osb[:, cc2 * 512 : (cc2 + 1) * 512]
                    if cc2 == 0:
                        nc.scalar.copy(dst, ps[:])
                    else:
                        nc.vector.tensor_copy(dst, ps[:])
                nc.sync.dma_start(out[qb * 128 : (qb + 1) * 128, :], osb[:])

        LOOKAHEAD = 4
        scores_done = 0
        v_done = 0
        pending_s4 = []

        def advance_scores(target):
            nonlocal scores_done, v_done
            while scores_done < min(target, len(units)):
                qc_next = units[scores_done][1]
                while v_done <= qc_next:
                    v_group(v_done)
                    v_done += 1
                emit_scores_pair(scores_done)
                scores_done += 2

        for i in range(len(units)):
            advance_scores(i + 1 + LOOKAHEAD)
            emit_pv(i)
            if pending_s4:
                emit_s4(pending_s4.pop())
            h, qc, kb, is_last = units[i]
            if is_last and h == NH - 1:
                if i == len(units) - 1:
                    emit_s4(qc)
                else:
                    pending_s4.append(qc)


_NC = None


def _build():
    global _NC
    if _NC is None:
        nc = bacc.Bacc("TRN2", target_bir_lowering=False)
        with tile.TileContext(nc) as tc:
            _body(tc)
        nc.compile()
        _NC = nc
    return _NC


def _shard_inputs(x, Wq, bq, Wk, bk, Wv, bv, Wp, bp):
    f = lambda a: np.ascontiguousarray(np.asarray(a, dtype=np.float32))
    zc = np.zeros(C, np.float32)
    in_maps = []
    for c in range(NCORES):
        b, hg = divmod(c, HG)
        cols = slice(hg * HS, (hg + 1) * HS)
        in_maps.append({
            "x": f(x[b]),
            "wq": f(Wq[:, cols]), "wk": f(Wk[:, cols]), "wv": f(Wv[:, cols]),
            "wp": f(Wp[cols, :]),
            "bq": f(bq[cols]), "bk": f(bk[cols]), "bv": f(bv[cols]),
            # bp is applied host-side during the unshard reduce
            "bp": zc,
        })
    return in_maps


def run_sharded(inputs, **run_kwargs):
    """Compile (cached), run on cores 0-7, gather. Returns (out, results)."""
    nc = _build()
    in_maps = _shard_inputs(**inputs)
    res = run_bass_kernel_spmd(nc, in_maps, core_ids=list(range(NCORES)), **run_kwargs)
    out = np.zeros((B, T, C), np.float32)
    for c in range(NCORES):
        b = c // HG
        out[b] += res.results[c]["out"]
    out += np.asarray(inputs["bp"], dtype=np.float32)
    return out, res


def kernel(x, Wq, bq, Wk, bk, Wv, bv, Wp, bp):
    out, _ = run_sharded(dict(
        x=x, Wq=Wq, bq=bq, Wk=Wk, bk=bk, Wv=Wv, bv=bv, Wp=Wp, bp=bp,
    ))
    return out
